# revision 4
# baseline (speedup 1.0000x reference)
"""Chamfer distance loss kernel for Trainium2 (8 NeuronCores).

Strategy
--------
reference: D[i,j] = ||pred_i - gt_j||^2 ; out = mean_i min_j D + mean_j min_i D.

We decompose into 8 independent jobs (4 batches x 2 directions), one per core.
For one job (query set A, candidate set B, both of size N=8192):

  * Host sorts A and B by x-coordinate.  For query rank i, the true nearest
    neighbor is almost always within a small rank window of i in the sorted
    B order.  Each 128-row query tile t scans the candidate window
    [128t - WL, 128t + SPAN - WL) (clamped via padding), SPAN wide.
  * The kernel computes, per query row, max_j (2<a,b_j> - ||b_j||^2) over the
    window via a K=4 TensorE matmul (features [2ax,2ay,2az,1] x [bx,by,bz,-||b||^2])
    and a VectorE free-axis max-reduce.  Then
    min_j D = ||a||^2 - rowmax, computed on host.
  * Exactness certificate (host): for query i with window [lo, hi), any
    excluded candidate j < lo has |a_x - b_x[j]| >= a_x - b_x[lo-1], so if
    band_min <= (x-margin)^2 on both sides the band min is the true min.
    The few rows that fail the certificate get an exact host-side scan.

Cores: core = 2*batch + direction (0: pred->gt, 1: gt->pred).
"""

import os

import numpy as np

import concourse.bass as bass
import concourse.tile as tile
from concourse import bacc, mybir
from concourse.bass_utils import run_bass_kernel_spmd

N = 8192  # points per cloud (both pred and gt)
B = 4  # batches
ROWT = 128  # query rows per tile
NTILES = N // ROWT  # 64
SPAN = 512  # candidate window width per row tile
WL = 192  # left extension of the window
WR = SPAN - WL - ROWT  # 192: right extension
PADDED = WL + N + WR  # padded candidate count
PAD_COORD = 1000.0  # sentinel coordinate for padding (never wins a min)

_CACHE = {}

# test.py introspection: set to BassKernelResults of the last run
LAST_RESULTS = None


def _build_program():
    nc = bacc.Bacc(
        "TRN2", target_bir_lowering=False, debug=False, num_devices=8
    )
    qfeat = nc.declare_dram_parameter(
        "qfeat", [4, N], mybir.dt.float32, isOutput=False
    )
    cfeat = nc.declare_dram_parameter(
        "cfeat", [4, PADDED], mybir.dt.float32, isOutput=False
    )
    rowmax_out = nc.declare_dram_parameter(
        "rowmax", [ROWT, NTILES], mybir.dt.float32, isOutput=True
    )

    with tile.TileContext(nc) as tc:
        with (
            tc.tile_pool(name="feats", bufs=1) as feats,
            tc.tile_pool(name="psum", bufs=8, space="PSUM") as psum_pool,
            tc.tile_pool(name="outp", bufs=1) as outp,
        ):
            q_sb = feats.tile([4, N], mybir.dt.float32, tag="q")
            nc.sync.dma_start(out=q_sb[:], in_=qfeat[:])
            c_sb = feats.tile([4, PADDED], mybir.dt.float32, tag="c")
            nc.sync.dma_start(out=c_sb[:], in_=cfeat[:])

            rmax = outp.tile([ROWT, NTILES], mybir.dt.float32)

            for t in range(NTILES):
                ps = psum_pool.tile([ROWT, SPAN], mybir.dt.float32)
                nc.tensor.matmul(
                    ps[:],
                    lhsT=q_sb[:, ROWT * t : ROWT * t + ROWT],
                    rhs=c_sb[:, ROWT * t : ROWT * t + SPAN],
                    start=True,
                    stop=True,
                )
                nc.vector.reduce_max(
                    rmax[:, t : t + 1], ps[:], axis=mybir.AxisListType.X
                )

            nc.sync.dma_start(out=rowmax_out[:], in_=rmax[:])
    nc.compile()
    return nc


def _job_arrays(A, Bset):
    """Build sorted feature arrays for one (query=A, candidate=B) job."""
    ao = np.argsort(A[:, 0], kind="stable")
    bo = np.argsort(Bset[:, 0], kind="stable")
    As = np.ascontiguousarray(A[ao])
    Bs = np.ascontiguousarray(Bset[bo])

    qfeat = np.empty((4, N), np.float32)
    qfeat[0:3] = (2.0 * As).T
    qfeat[3] = 1.0

    cfeat = np.empty((4, PADDED), np.float32)
    cfeat[0:3] = PAD_COORD
    cfeat[3] = -3.0 * PAD_COORD * PAD_COORD
    cfeat[0:3, WL : WL + N] = Bs.T
    cfeat[3, WL : WL + N] = -(Bs.astype(np.float64) ** 2).sum(1).astype(np.float32)
    return As, Bs, qfeat, cfeat


def kernel(pred: np.ndarray, gt: np.ndarray) -> np.ndarray:
    global LAST_RESULTS
    pred = np.asarray(pred, dtype=np.float32)
    gt = np.asarray(gt, dtype=np.float32)
    assert pred.shape == (B, N, 3) and gt.shape == (B, N, 3)

    if "nc" not in _CACHE:
        _CACHE["nc"] = _build_program()
    nc = _CACHE["nc"]

    jobs = []
    in_maps = []
    for b in range(B):
        for A, Bset in ((pred[b], gt[b]), (gt[b], pred[b])):
            As, Bs, qfeat, cfeat = _job_arrays(A, Bset)
            jobs.append((As, Bs))
            in_maps.append({"qfeat": qfeat, "cfeat": cfeat})

    trace = bool(int(os.environ.get("CHAMFER_TRACE", "0")))
    bk = run_bass_kernel_spmd(nc, in_maps, list(range(8)), trace=trace)
    LAST_RESULTS = bk
    results = bk.results

    # Host: undo the rowmax formulation, certify, fix up, and average.
    total = 0.0
    i = np.arange(N)
    t = i // ROWT
    lo = ROWT * t - WL  # window start (unpadded coords, may be < 0)
    hi = ROWT * t + (SPAN - WL)  # window end (may be > N)
    for (As, Bs), r in zip(jobs, results):
        rowmax = np.asarray(r["rowmax"])  # [128, 64]
        asq = (As.astype(np.float64) ** 2).sum(1)
        d_band = asq - rowmax.T.reshape(-1).astype(np.float64)

        bx = Bs[:, 0].astype(np.float64)
        ax = As[:, 0].astype(np.float64)
        lmarg = np.where(lo >= 1, ax - bx[np.clip(lo - 1, 0, N - 1)], np.inf)
        rmarg = np.where(hi < N, bx[np.clip(hi, 0, N - 1)] - ax, np.inf)
        marg = np.minimum(lmarg, rmarg)
        ok = (marg >= 0) & (d_band <= marg * marg)
        bad = np.flatnonzero(~ok)
        if bad.size:
            Ad = As[bad].astype(np.float64)
            Bd = Bs.astype(np.float64)
            d = ((Ad[:, None, :] - Bd[None, :, :]) ** 2).sum(-1)
            d_band[bad] = d.min(1)
        total += d_band.mean()

    return np.float32(total / B)
